# revision 73
# baseline (speedup 1.0000x reference)
"""Single-head causal attention (B=1024,T=256,C=512,H=64), data-parallel on 8 TRN2 cores.

Host prep: x is cast to bf16 and pre-transposed to x^T [B, C, T]; weights packed
as wqk = [Wq|Wk] ([512,128]) and wv ([512,64]) in bf16. This removes all PE
transposes of x and the on-device f32->bf16 casts, and halves HBM traffic.

Per core: nb=128 batches processed in PAIRS. Per pair (all layouts chosen so no
PE transpose is ever needed):
  xt     = x^T [128(c_j), 256(t)] chunks, one 256KB DMA per batch
  qkT    = wqk_j^T @ xt_j  (acc over j, both batches) ([128(q|k=h), 512(tA|tB)] PSUM)
  qk_sb  = cast(qkT)  (Act)   kt = [kT ; 0] [128(h-pad), 512(s)]  (DVE, pre-zeroed rows)
Per batch:
  v[tt]  = xt_j[:,tt]^T @ wv_j (acc over j)   (natural [128(t), 64] PSUM)
  v1[tt] = [v | 1]   [128(s), 65]             (DVE copy into pre-set ones col)
  weiT   = kt-chunk^T @ qk_sb-half            (zero rows kill the k-row contribution)
  e      = exp(0.125 * weiT)  (one Act instr, bf16), causal affine_select (t>=s)
  out    = e-chunk^T @ v1     (natural [128(t), 65]; col 64 = softmax denom)
  y      = out[:, 0:64] * (1/out[:, 64])      (DVE rcp, Act mul), bf16 store, one DMA
"""

import sys, json

for _p in ("/opt/trn_rl_repo", "/root/.axon_site/_ro/trn_rl_repo"):
    if _p not in sys.path:
        sys.path.append(_p)

import numpy as np
import ml_dtypes
import concourse.bass as bass
import concourse.tile as tile
from concourse import mybir
from concourse.bass_utils import run_bass_kernel_spmd

N_CORES = 8
B, T, C, H = 1024, 256, 512, 64
NB = B // N_CORES  # batches per core
CD = mybir.dt.bfloat16
F32 = mybir.dt.float32
BF16 = ml_dtypes.bfloat16

_MAX_CTRL_WAITS = 1


def _patch_waits(nc):
    """walrus on this toolchain rejects >1 sync-wait on TPB_CTRL (NoOp/Drain/
    EventSemaphore) instructions; hoist excess waits into preceding NoOps."""
    raw = type(nc).to_json_bytes(nc)
    j = json.loads(raw)
    ctr = 0
    for f in j.get("functions", []):
        for bb in f.get("basicblocks", f.get("blocks", [])):
            out = []
            for i in bb.get("instructions", []):
                si = i.get("sync_info") or {}
                ow = si.get("on_wait") or []
                has_update = bool((si.get("on_update") or []))
                splittable = i.get("opcode") != "EventSemaphore" or not has_update
                if len(ow) > _MAX_CTRL_WAITS and splittable:
                    excess, keep = ow[:-_MAX_CTRL_WAITS], ow[-_MAX_CTRL_WAITS:]
                    while excess:
                        chunk, excess = excess[:_MAX_CTRL_WAITS], excess[_MAX_CTRL_WAITS:]
                        ctr += 1
                        out.append({
                            "name": f"WSPLIT-{ctr}",
                            "opcode": "NoOp",
                            "engine": i["engine"],
                            "ins": [], "outs": [],
                            "debug": i.get("debug", 0),
                            "sync_info": {"on_wait": chunk, "on_update": []},
                        })
                    si["on_wait"] = keep
                    i["sync_info"] = si
                out.append(i)
            bb["instructions"] = out
    data = json.dumps(j).encode()
    nc.to_json_bytes = lambda: data
    return nc


def build(nb=NB):
    assert nb % 2 == 0
    nc = bass.Bass("TRN2", target_bir_lowering=False, debug=False, enable_asserts=False)
    # x^T prepacked on host per batch-PAIR: rows ordered (j-chunk, batch-half,
    # partition) so one 3D-AP DMA drops the pair into the j-outer tile layout
    xt_d = nc.dram_tensor("xt", [nb // 2, 2 * C, T], CD, kind="ExternalInput").ap()
    wqk_d = nc.dram_tensor("wqk", [C, 2 * H], CD, kind="ExternalInput").ap()
    wv_d = nc.dram_tensor("wv", [C, H], CD, kind="ExternalInput").ap()
    # y packed per batch-PAIR: rows (h, tt, p) -> host just reshapes to [nb, T, H]
    y_d = nc.dram_tensor("y", [nb // 2, 2 * T, H], CD, kind="ExternalOutput").ap()

    DEP = 8  # rotation depth for the persistent kt / v1 tiles

    with tile.TileContext(nc) as tc:
        with (
            tc.tile_pool(name="consts", bufs=1) as consts,
            tc.tile_pool(name="xt", bufs=5) as p_xt,
            tc.tile_pool(name="qksb", bufs=8) as p_qksb,
            tc.tile_pool(name="esb", bufs=10) as p_e,
            tc.tile_pool(name="ysb", bufs=10) as p_y,
            tc.tile_pool(name="osb", bufs=12) as p_osb,
            tc.tile_pool(name="qkps", bufs=2, space="PSUM") as p_qk,
            tc.tile_pool(name="wps", bufs=2, space="PSUM") as p_w,
            tc.tile_pool(name="vops", bufs=4, space="PSUM") as p_vo,
        ):
            # ---- constants ----
            wqk_sb = consts.tile([128, 512], CD, name="wqk_sb")
            wv_sb = consts.tile([128, 256], CD, name="wv_sb")
            for j in range(4):
                nc.sync.dma_start(wqk_sb[:, 128 * j : 128 * (j + 1)], wqk_d[128 * j : 128 * (j + 1), :])
                nc.sync.dma_start(wv_sb[:, 64 * j : 64 * (j + 1)], wv_d[128 * j : 128 * (j + 1), :])

            # kt tiles (one per batch-PAIR): rows 0:64 get kT, rows 64:128 stay zero
            kt_tiles = []
            for i in range(DEP):
                kt = consts.tile([128, 512], CD, name=f"kt{i}")
                nc.gpsimd.memset(kt[64:128, :], 0.0)
                kt_tiles.append(kt)
            # v1 tiles [v0 | 1 | v1 | 1]: cols 64 & 129 stay 1.0 forever
            # (softmax denominator trick)
            v1_tiles = []
            for i in range(2 * DEP):
                v1p = consts.tile([128, 130], CD, name=f"v1_{i}")
                nc.gpsimd.memset(v1p[:, 64:65], 1.0)
                nc.gpsimd.memset(v1p[:, 129:130], 1.0)
                v1_tiles.append(v1p)

            for bp in range(nb // 2):  # batch pairs
                kt = kt_tiles[bp % DEP]
                # ---- load x^T for the pair: ONE 512KB DMA; j-outer layout:
                # chunk j at cols 512j, batch h at 512j+256h ----
                xt_pair = p_xt.tile([128, 2048], CD, tag="xt", name="xtpair")
                nc.sync.dma_start(
                    xt_pair[:].rearrange("p (jh t) -> p jh t", jh=8),
                    xt_d[bp].rearrange("(jh p) t -> p jh t", jh=8),
                )

                # ---- qkT for the pair: [128(q|k), 512(tA|tB)] (one full bank,
                # single accumulation group, 4 LDW total) ----
                qk_ps = p_qk.tile([128, 512], F32, tag="qkps", name="qkp")
                for j in range(4):
                    nc.tensor.matmul(
                        qk_ps[:],
                        wqk_sb[:, 128 * j : 128 * (j + 1)],
                        xt_pair[:, 512 * j : 512 * (j + 1)],
                        start=(j == 0), stop=(j == 3),
                    )

                # ---- pair-wide PSUM -> SBUF casts (kt alternates DVE/Act
                # to balance the two elementwise engines). High priority:
                # both feed PE operands (wei moving + stationary) ----
                qk_sb = p_qksb.tile([128, 512], CD, tag="qksb", name="qksb")
                with tc.high_priority(offset=40):
                    nc.scalar.copy(qk_sb[:], qk_ps[:])                      # Act
                    if bp % 2 == 0:
                        nc.vector.tensor_copy(kt[0:64, :], qk_ps[64:128, :])  # DVE
                    else:
                        nc.scalar.copy(kt[0:64, :], qk_ps[64:128, :])         # Act

                y_sb = p_y.tile([128, 256], CD, tag="ysb", name="ysb")
                for h in range(2):
                    b = 2 * bp + h
                    v1p = v1_tiles[b % (2 * DEP)]

                    # ---- v natural + out share one PSUM bank ----
                    vo = p_vo.tile([128, 258], F32, tag="vops", name="vo")
                    v_ps = [vo[:, 0:64], vo[:, 64:128]]
                    o_ps = [vo[:, 128:193], vo[:, 193:258]]
                    for tt in range(2):
                        for j in range(4):
                            base_c = 512 * j + 256 * h + 128 * tt
                            nc.tensor.matmul(
                                v_ps[tt],
                                xt_pair[:, base_c : base_c + 128],
                                wv_sb[:, 64 * j : 64 * (j + 1)],
                                start=(j == 0), stop=(j == 3),
                            )
                    # one strided cast fills both v blocks (ones cols untouched);
                    # high priority: feeds the attout moving operand
                    with tc.high_priority(offset=40):
                        nc.vector.tensor_copy(
                            v1p[:].rearrange("p (tt c) -> p tt c", c=65)[:, :, 0:64],
                            vo[:, 0:128].rearrange("p (tt c) -> p tt c", c=64),
                        )

                    # ---- weiT = k q^T (padded-k stationary, this batch's halves) ----
                    w_ps = p_w.tile([128, 384], F32, tag="wps", name="wp")
                    nc.tensor.matmul(
                        w_ps[:, 0:256], kt[:, 256 * h : 256 * h + 128],
                        qk_sb[:, 256 * h : 256 * (h + 1)], start=True, stop=True,
                    )
                    nc.tensor.matmul(
                        w_ps[:, 256:384], kt[:, 256 * h + 128 : 256 * (h + 1)],
                        qk_sb[:, 256 * h + 128 : 256 * (h + 1)], start=True, stop=True,
                    )

                    # ---- exp (single instr) + causal mask (one strided instr
                    # covers blocks 0:128 and 256:384) ----
                    e = p_e.tile([128, 384], CD, tag="esb", name="esb")
                    # exp gates the attout weight-loads (measured PE stall);
                    # raise its scheduler priority over the next pair's Act copies
                    with tc.high_priority(offset=40):
                        nc.scalar.activation(e[:], w_ps[:], mybir.ActivationFunctionType.Exp, scale=0.125)
                    e_blocks = e[:].rearrange("p (bk c) -> p bk c", c=128)[:, 0:3:2, :]
                    with tc.high_priority(offset=40):
                        nc.gpsimd.affine_select(
                            out=e_blocks, in_=e_blocks, compare_op=mybir.AluOpType.is_ge,
                            fill=0.0, base=0, pattern=[[0, 2], [1, 128]], channel_multiplier=-1,
                        )

                    # ---- out natural [128(t), 65]; col 64 = denom ----
                    nc.tensor.matmul(o_ps[0], e[:, 0:128], v1p[:, 0:65], start=True, stop=True)
                    nc.tensor.matmul(o_ps[1], e[:, 128:256], v1p[:, 0:65], start=True, stop=False)
                    nc.tensor.matmul(o_ps[1], e[:, 256:384], v1p[:, 65:130], start=False, stop=True)

                    # ---- normalize: one paired rcp + ONE fused broadcast-mul ----
                    rcp2 = p_osb.tile([128, 2], F32, tag="osb", name="rcp2")
                    nc.vector.reciprocal(rcp2[:], vo[:, 192:258:65])
                    o3 = vo[:, 128:258].rearrange("p (b c) -> p b c", c=65)[:, :, 0:64]
                    y3 = y_sb[:, 128 * h : 128 * (h + 1)].rearrange("p (b c) -> p b c", c=64)
                    r3 = rcp2[:].rearrange("p (a b) -> p a b", b=1)
                    _, rb = bass.broadcast_tensor_aps(o3, r3)
                    nc.vector.tensor_mul(y3, o3, rb)

                # ---- one y DMA per pair ----
                nc.sync.dma_start(
                    y_d[bp].rearrange("(x p) h -> p x h", x=4),
                    y_sb[:].rearrange("p (x h) -> p x h", x=4),
                )

    return _patch_waits(nc)


_CACHED = {}


def _get_nc(nb=NB):
    if nb not in _CACHED:
        _CACHED[nb] = build(nb)
    return _CACHED[nb]


def kernel(x, Wq, Wk, Wv, _nc=None, _trace=False, _tmpdir=None):
    x = np.asarray(x)
    nb = x.shape[0] // N_CORES
    nc = _nc if _nc is not None else _get_nc(nb)
    # host-side prep: bf16 cast + transpose to x^T, then pack batch pairs with
    # c-chunk outer: [B/2, (4j, 2h, 128p), T]
    xt = np.ascontiguousarray(
        x.astype(BF16)
        .transpose(0, 2, 1)
        .reshape(x.shape[0] // 2, 2, 4, 128, T)
        .swapaxes(1, 2)
        .reshape(x.shape[0] // 2, 2 * C, T)
    )
    wqk = np.ascontiguousarray(np.concatenate([np.asarray(Wq), np.asarray(Wk)], axis=1).astype(BF16))
    wv = np.ascontiguousarray(np.asarray(Wv).astype(BF16))
    in_maps = [
        {"xt": xt[i * nb // 2 : (i + 1) * nb // 2], "wqk": wqk, "wv": wv}
        for i in range(N_CORES)
    ]
    res = run_bass_kernel_spmd(
        nc, in_maps, core_ids=list(range(N_CORES)), trace=_trace, tmpdir=_tmpdir
    )
    out = np.concatenate(
        [res.results[i]["y"].reshape(nb, T, H) for i in range(N_CORES)], axis=0
    ).astype(np.float32)
    if _trace:
        kernel.last_results = res
    return out


# revision 74
# speedup vs baseline: 1.0073x; 1.0073x over previous
"""Single-head causal attention (B=1024,T=256,C=512,H=64), data-parallel on 8 TRN2 cores.

Host prep: x is cast to bf16 and pre-transposed to x^T [B, C, T]; weights packed
as wqk = [Wq|Wk] ([512,128]) and wv ([512,64]) in bf16. This removes all PE
transposes of x and the on-device f32->bf16 casts, and halves HBM traffic.

Per core: nb=128 batches processed in PAIRS. Per pair (all layouts chosen so no
PE transpose is ever needed):
  xt     = x^T [128(c_j), 256(t)] chunks, one 256KB DMA per batch
  qkT    = wqk_j^T @ xt_j  (acc over j, both batches) ([128(q|k=h), 512(tA|tB)] PSUM)
  qk_sb  = cast(qkT)  (Act)   kt = [kT ; 0] [128(h-pad), 512(s)]  (DVE, pre-zeroed rows)
Per batch:
  v[tt]  = xt_j[:,tt]^T @ wv_j (acc over j)   (natural [128(t), 64] PSUM)
  v1[tt] = [v | 1]   [128(s), 65]             (DVE copy into pre-set ones col)
  weiT   = kt-chunk^T @ qk_sb-half            (zero rows kill the k-row contribution)
  e      = exp(0.125 * weiT)  (one Act instr, bf16), causal affine_select (t>=s)
  out    = e-chunk^T @ v1     (natural [128(t), 65]; col 64 = softmax denom)
  y      = out[:, 0:64] * (1/out[:, 64])      (DVE rcp, Act mul), bf16 store, one DMA
"""

import sys, json

for _p in ("/opt/trn_rl_repo", "/root/.axon_site/_ro/trn_rl_repo"):
    if _p not in sys.path:
        sys.path.append(_p)

import numpy as np
import ml_dtypes
import concourse.bass as bass
import concourse.tile as tile
from concourse import mybir
from concourse.bass_utils import run_bass_kernel_spmd

N_CORES = 8
B, T, C, H = 1024, 256, 512, 64
NB = B // N_CORES  # batches per core
CD = mybir.dt.bfloat16
F32 = mybir.dt.float32
BF16 = ml_dtypes.bfloat16

_MAX_CTRL_WAITS = 1


def _patch_waits(nc):
    """walrus on this toolchain rejects >1 sync-wait on TPB_CTRL (NoOp/Drain/
    EventSemaphore) instructions; hoist excess waits into preceding NoOps."""
    raw = type(nc).to_json_bytes(nc)
    j = json.loads(raw)
    ctr = 0
    for f in j.get("functions", []):
        for bb in f.get("basicblocks", f.get("blocks", [])):
            out = []
            for i in bb.get("instructions", []):
                si = i.get("sync_info") or {}
                ow = si.get("on_wait") or []
                has_update = bool((si.get("on_update") or []))
                splittable = i.get("opcode") != "EventSemaphore" or not has_update
                if len(ow) > _MAX_CTRL_WAITS and splittable:
                    excess, keep = ow[:-_MAX_CTRL_WAITS], ow[-_MAX_CTRL_WAITS:]
                    while excess:
                        chunk, excess = excess[:_MAX_CTRL_WAITS], excess[_MAX_CTRL_WAITS:]
                        ctr += 1
                        out.append({
                            "name": f"WSPLIT-{ctr}",
                            "opcode": "NoOp",
                            "engine": i["engine"],
                            "ins": [], "outs": [],
                            "debug": i.get("debug", 0),
                            "sync_info": {"on_wait": chunk, "on_update": []},
                        })
                    si["on_wait"] = keep
                    i["sync_info"] = si
                out.append(i)
            bb["instructions"] = out
    data = json.dumps(j).encode()
    nc.to_json_bytes = lambda: data
    return nc


def build(nb=NB):
    assert nb % 2 == 0
    nc = bass.Bass("TRN2", target_bir_lowering=False, debug=False, enable_asserts=False)
    # x^T prepacked on host per batch-PAIR: rows ordered (j-chunk, batch-half,
    # partition) so one 3D-AP DMA drops the pair into the j-outer tile layout
    xt_d = nc.dram_tensor("xt", [nb // 2, 2 * C, T], CD, kind="ExternalInput").ap()
    wqk_d = nc.dram_tensor("wqk", [C, 2 * H], CD, kind="ExternalInput").ap()
    wv_d = nc.dram_tensor("wv", [C, H], CD, kind="ExternalInput").ap()
    # y packed per batch-PAIR: rows (h, tt, p) -> host just reshapes to [nb, T, H]
    y_d = nc.dram_tensor("y", [nb // 2, 2 * T, H], CD, kind="ExternalOutput").ap()

    DEP = 8  # rotation depth for the persistent kt / v1 tiles

    with tile.TileContext(nc) as tc:
        with (
            tc.tile_pool(name="consts", bufs=1) as consts,
            tc.tile_pool(name="xt", bufs=5) as p_xt,
            tc.tile_pool(name="qksb", bufs=8) as p_qksb,
            tc.tile_pool(name="esb", bufs=10) as p_e,
            tc.tile_pool(name="ysb", bufs=10) as p_y,
            tc.tile_pool(name="osb", bufs=12) as p_osb,
            tc.tile_pool(name="qkps", bufs=2, space="PSUM") as p_qk,
            tc.tile_pool(name="wps", bufs=2, space="PSUM") as p_w,
            tc.tile_pool(name="vops", bufs=4, space="PSUM") as p_vo,
        ):
            # ---- constants ----
            wqk_sb = consts.tile([128, 512], CD, name="wqk_sb")
            wv_sb = consts.tile([128, 256], CD, name="wv_sb")
            for j in range(4):
                nc.sync.dma_start(wqk_sb[:, 128 * j : 128 * (j + 1)], wqk_d[128 * j : 128 * (j + 1), :])
                nc.sync.dma_start(wv_sb[:, 64 * j : 64 * (j + 1)], wv_d[128 * j : 128 * (j + 1), :])

            # kt tiles (one per batch-PAIR): rows 0:64 get kT, rows 64:128 stay zero
            kt_tiles = []
            for i in range(DEP):
                kt = consts.tile([128, 512], CD, name=f"kt{i}")
                nc.gpsimd.memset(kt[64:128, :], 0.0)
                kt_tiles.append(kt)
            # v1 tiles [v0 | 1 | v1 | 1]: cols 64 & 129 stay 1.0 forever
            # (softmax denominator trick)
            v1_tiles = []
            for i in range(2 * DEP):
                v1p = consts.tile([128, 130], CD, name=f"v1_{i}")
                nc.gpsimd.memset(v1p[:, 64:65], 1.0)
                nc.gpsimd.memset(v1p[:, 129:130], 1.0)
                v1_tiles.append(v1p)

            for bp in range(nb // 2):  # batch pairs
                kt = kt_tiles[bp % DEP]
                # ---- load x^T for the pair: ONE 512KB DMA; j-outer layout:
                # chunk j at cols 512j, batch h at 512j+256h ----
                xt_pair = p_xt.tile([128, 2048], CD, tag="xt", name="xtpair")
                # x feeds everything: keep its triggers ahead of y stores on SP
                with tc.high_priority(offset=40):
                    nc.sync.dma_start(
                        xt_pair[:].rearrange("p (jh t) -> p jh t", jh=8),
                        xt_d[bp].rearrange("(jh p) t -> p jh t", jh=8),
                    )

                # ---- qkT for the pair: [128(q|k), 512(tA|tB)] (one full bank,
                # single accumulation group, 4 LDW total) ----
                qk_ps = p_qk.tile([128, 512], F32, tag="qkps", name="qkp")
                for j in range(4):
                    nc.tensor.matmul(
                        qk_ps[:],
                        wqk_sb[:, 128 * j : 128 * (j + 1)],
                        xt_pair[:, 512 * j : 512 * (j + 1)],
                        start=(j == 0), stop=(j == 3),
                    )

                # ---- pair-wide PSUM -> SBUF casts (kt alternates DVE/Act
                # to balance the two elementwise engines). High priority:
                # both feed PE operands (wei moving + stationary) ----
                qk_sb = p_qksb.tile([128, 512], CD, tag="qksb", name="qksb")
                with tc.high_priority(offset=40):
                    nc.scalar.copy(qk_sb[:], qk_ps[:])                      # Act
                    if bp % 2 == 0:
                        nc.vector.tensor_copy(kt[0:64, :], qk_ps[64:128, :])  # DVE
                    else:
                        nc.scalar.copy(kt[0:64, :], qk_ps[64:128, :])         # Act

                y_sb = p_y.tile([128, 256], CD, tag="ysb", name="ysb")
                for h in range(2):
                    b = 2 * bp + h
                    v1p = v1_tiles[b % (2 * DEP)]

                    # ---- v natural + out share one PSUM bank ----
                    vo = p_vo.tile([128, 258], F32, tag="vops", name="vo")
                    v_ps = [vo[:, 0:64], vo[:, 64:128]]
                    o_ps = [vo[:, 128:193], vo[:, 193:258]]
                    for tt in range(2):
                        for j in range(4):
                            base_c = 512 * j + 256 * h + 128 * tt
                            nc.tensor.matmul(
                                v_ps[tt],
                                xt_pair[:, base_c : base_c + 128],
                                wv_sb[:, 64 * j : 64 * (j + 1)],
                                start=(j == 0), stop=(j == 3),
                            )
                    # one strided cast fills both v blocks (ones cols untouched);
                    # high priority: feeds the attout moving operand
                    with tc.high_priority(offset=40):
                        nc.vector.tensor_copy(
                            v1p[:].rearrange("p (tt c) -> p tt c", c=65)[:, :, 0:64],
                            vo[:, 0:128].rearrange("p (tt c) -> p tt c", c=64),
                        )

                    # ---- weiT = k q^T (padded-k stationary, this batch's halves) ----
                    w_ps = p_w.tile([128, 384], F32, tag="wps", name="wp")
                    nc.tensor.matmul(
                        w_ps[:, 0:256], kt[:, 256 * h : 256 * h + 128],
                        qk_sb[:, 256 * h : 256 * (h + 1)], start=True, stop=True,
                    )
                    nc.tensor.matmul(
                        w_ps[:, 256:384], kt[:, 256 * h + 128 : 256 * (h + 1)],
                        qk_sb[:, 256 * h + 128 : 256 * (h + 1)], start=True, stop=True,
                    )

                    # ---- exp (single instr) + causal mask (one strided instr
                    # covers blocks 0:128 and 256:384) ----
                    e = p_e.tile([128, 384], CD, tag="esb", name="esb")
                    # exp gates the attout weight-loads (measured PE stall);
                    # raise its scheduler priority over the next pair's Act copies
                    with tc.high_priority(offset=40):
                        nc.scalar.activation(e[:], w_ps[:], mybir.ActivationFunctionType.Exp, scale=0.125)
                    e_blocks = e[:].rearrange("p (bk c) -> p bk c", c=128)[:, 0:3:2, :]
                    with tc.high_priority(offset=40):
                        nc.gpsimd.affine_select(
                            out=e_blocks, in_=e_blocks, compare_op=mybir.AluOpType.is_ge,
                            fill=0.0, base=0, pattern=[[0, 2], [1, 128]], channel_multiplier=-1,
                        )

                    # ---- out natural [128(t), 65]; col 64 = denom ----
                    nc.tensor.matmul(o_ps[0], e[:, 0:128], v1p[:, 0:65], start=True, stop=True)
                    nc.tensor.matmul(o_ps[1], e[:, 128:256], v1p[:, 0:65], start=True, stop=False)
                    nc.tensor.matmul(o_ps[1], e[:, 256:384], v1p[:, 65:130], start=False, stop=True)

                    # ---- normalize: one paired rcp + ONE fused broadcast-mul ----
                    rcp2 = p_osb.tile([128, 2], F32, tag="osb", name="rcp2")
                    nc.vector.reciprocal(rcp2[:], vo[:, 192:258:65])
                    o3 = vo[:, 128:258].rearrange("p (b c) -> p b c", c=65)[:, :, 0:64]
                    y3 = y_sb[:, 128 * h : 128 * (h + 1)].rearrange("p (b c) -> p b c", c=64)
                    r3 = rcp2[:].rearrange("p (a b) -> p a b", b=1)
                    _, rb = bass.broadcast_tensor_aps(o3, r3)
                    nc.vector.tensor_mul(y3, o3, rb)

                # ---- one y DMA per pair ----
                nc.sync.dma_start(
                    y_d[bp].rearrange("(x p) h -> p x h", x=4),
                    y_sb[:].rearrange("p (x h) -> p x h", x=4),
                )

    return _patch_waits(nc)


_CACHED = {}


def _get_nc(nb=NB):
    if nb not in _CACHED:
        _CACHED[nb] = build(nb)
    return _CACHED[nb]


def kernel(x, Wq, Wk, Wv, _nc=None, _trace=False, _tmpdir=None):
    x = np.asarray(x)
    nb = x.shape[0] // N_CORES
    nc = _nc if _nc is not None else _get_nc(nb)
    # host-side prep: bf16 cast + transpose to x^T, then pack batch pairs with
    # c-chunk outer: [B/2, (4j, 2h, 128p), T]
    xt = np.ascontiguousarray(
        x.astype(BF16)
        .transpose(0, 2, 1)
        .reshape(x.shape[0] // 2, 2, 4, 128, T)
        .swapaxes(1, 2)
        .reshape(x.shape[0] // 2, 2 * C, T)
    )
    wqk = np.ascontiguousarray(np.concatenate([np.asarray(Wq), np.asarray(Wk)], axis=1).astype(BF16))
    wv = np.ascontiguousarray(np.asarray(Wv).astype(BF16))
    in_maps = [
        {"xt": xt[i * nb // 2 : (i + 1) * nb // 2], "wqk": wqk, "wv": wv}
        for i in range(N_CORES)
    ]
    res = run_bass_kernel_spmd(
        nc, in_maps, core_ids=list(range(N_CORES)), trace=_trace, tmpdir=_tmpdir
    )
    out = np.concatenate(
        [res.results[i]["y"].reshape(nb, T, H) for i in range(N_CORES)], axis=0
    ).astype(np.float32)
    if _trace:
        kernel.last_results = res
    return out
